# revision 6
# baseline (speedup 1.0000x reference)
"""Non-local (spatial self-attention) denoising block on 8 Trainium2 cores.

Reference math (per sample n, with x:[C,HW], D=C/2):
    t = (W_theta @ x + b_theta) / sqrt(D)      [D, HW]
    p = W_phi   @ x + b_phi                    [D, HW]
    S[q,k] = t[:,q] . p[:,k]
    f = softmax_k(S)
    attn = x @ f.T
    out = x + W_fuse @ attn + b_fuse

Device formulation -- all matmul phases run fp8e4 (e4m3) DoubleRow, which
packs two 128-row k-tiles per instruction (~1.9x bf16 PE throughput):
    Sᵀ[k,q] = p.T @ t    -- keys on partitions so no transposes are needed
    e = exp(Sᵀ/1024 - 3) -- shift is softmax-invariant; keeps e in e4m3 range
    G'ᵀ = xᵀ @ (16·W_fuseᵀ)  -- output conv fused into the values: [HW, C]
    Zb = (16·ones)ᵀ @ e  -- Z on all 128 partitions via accumulating DR
                            matmuls over the 8 e tiles (replaces a 7-op DVE
                            f32 add chain that was the v1 bottleneck)
    y = G'ᵀᵀ @ e          -- [C, HW] unnormalized
    out = y * (1/Zb) + (x + b_fuse)

fp8 scale management (e4m3: max 240, min normal 2^-7): W_theta/W_phi are
boosted x8 host-side (raw 0.05-scale weights would sit in subnormals),
W_fuse x16; the 1/sqrt(D) softmax scale and both theta/phi boosts fold into
the exp activation's scale (1/1024), the W_fuse boost into the Z ones.

Schedule (per sample, steady state): conv -> kp-loop{S(2kp), S(2kp+1),
gt-pair(kp), y0(kp), y1(kp), Z(kp)} -> y2, y3 -> combine. y0/y1 accumulate
in a RESIDENT psum pool across the kp loop so the y matmuls consume e tiles
as ACT produces them -- no exp-lag stall before the y phase. PSUM: resident
pool (y0,y1,y3) 4 banks + rotating pool (conv/S/gt/Z/y2) 4 banks.
Engine balance per sample: PE ~21us, ACT ~11.5us (exps + half the conv
copies), DVE ~16us (other copies, gt copies, recip, final mul/add).

Sharding: data-parallel over batch N=32 -> 4 samples per core on 8 cores.
A ~3.8us burst of junk matmuls pre-warms the PE clock (HAM) while the first
DMAs land; a dummy exp pre-loads the ACT spline table.
"""

import numpy as np
import ml_dtypes

import concourse.bass as bass
import concourse.tile as tile
from concourse import bacc, mybir
from concourse import bass_utils

F32 = mybir.dt.float32
BF16 = mybir.dt.bfloat16
F8 = mybir.dt.float8e4
AF = mybir.ActivationFunctionType
DR = mybir.MatmulPerfMode.DoubleRow

N, C, H, W = 32, 512, 32, 32
D = C // 2
HW = H * W
NCORES = 8
NS = N // NCORES  # samples per core
P = 128
CT = C // P   # 4 c-tiles
KT = HW // P  # 8 hw-tiles
MT_D = (2 * D) // P  # 4 m-tiles of combined theta/phi conv
NQ = HW // 512  # 2 free-dim halves
KP = KT // 2  # 4 k-pairs for DoubleRow over HW

TP_BOOST = 8.0     # on W_theta and W_phi (and their biases)
FU_BOOST = 16.0    # on W_fuse
EXP_SCALE = 1.0 / (TP_BOOST * TP_BOOST * np.sqrt(np.float32(D)))
EXP_BIAS = -3.0    # softmax-shift: keeps exp() within e4m3 range


def _emit(tc):
    nc = tc.nc

    x_f8 = nc.dram_tensor("x_f8", [NS, C, HW], F8, kind="ExternalInput").ap()
    x_res = nc.dram_tensor("x_res", [NS, C, HW], BF16, kind="ExternalInput").ap()
    wcat_t = nc.dram_tensor("wcat_t", [C, 2 * D], F8, kind="ExternalInput").ap()
    b_cat = nc.dram_tensor("b_cat", [2 * D, 1], F32, kind="ExternalInput").ap()
    wfu_t = nc.dram_tensor("wfu_t", [C, C], F8, kind="ExternalInput").ap()
    out_d = nc.dram_tensor("out", [NS, C, HW], BF16, kind="ExternalOutput").ap()

    import contextlib
    ctx = contextlib.ExitStack()
    with ctx:
        # ---- constant pools ----
        wpool = ctx.enter_context(tc.tile_pool(name="wpool", bufs=1))
        wcat_sb = wpool.tile([P, CT, 2 * D], F8)

        # ---- working pools ----
        xf8_pool = ctx.enter_context(tc.tile_pool(name="xf8", bufs=3))
        xres_pool = ctx.enter_context(tc.tile_pool(name="xres", bufs=2))
        tp_pool = ctx.enter_context(tc.tile_pool(name="tp", bufs=2))
        gt_pool = ctx.enter_context(tc.tile_pool(name="gt", bufs=2))
        e_pool = ctx.enter_context(tc.tile_pool(name="e", bufs=2))
        rz_pool = ctx.enter_context(tc.tile_pool(name="rz", bufs=2))
        fin_pool = ctx.enter_context(tc.tile_pool(name="fin", bufs=3))
        out_pool = ctx.enter_context(tc.tile_pool(name="outp", bufs=3))

        # PSUM: resident pool for the kp-loop-accumulating y0/y1 (+y3),
        # rotating pool for everything else. 2x2 + 2x2 banks = all 8.
        psum_y = ctx.enter_context(tc.tile_pool(name="psy", bufs=2, space="PSUM"))
        psum_rot = ctx.enter_context(tc.tile_pool(name="psr", bufs=2, space="PSUM"))

        # HAM pre-warm + ACT exp-table pre-load during the initial DMA wait
        ones_f8 = wpool.tile([P, 2, P], F8)
        nc.vector.memset(ones_f8[:], FU_BOOST)  # folds the wfu boost out of Z
        warm_rhs = wpool.tile([P, 512], BF16)
        nc.vector.memset(warm_rhs[:], 0.0)
        ebias_sb = wpool.tile([P, 1], F32)
        nc.vector.memset(ebias_sb[:], EXP_BIAS)
        warm_e = wpool.tile([P, 16], F8)
        nc.scalar.activation(warm_e[:], warm_rhs[:, 0:16], AF.Exp,
                             bias=ebias_sb[:], scale=float(EXP_SCALE))
        ps_warm = psum_rot.tile([P, 512], F32, tag="mm", name="ps_warm")
        warm_lhs = wpool.tile([P, P], BF16)
        nc.vector.memset(warm_lhs[:], 0.0)
        for w in range(9):
            nc.tensor.matmul(ps_warm[:], warm_lhs[:], warm_rhs[:],
                             start=True, stop=True)

        xf8_tiles = {}
        xf8_tiles[0] = xf8_pool.tile([P, CT, HW], F8, tag="xf8", name="xf80")
        for k in range(CT):
            nc.sync.dma_start(
                wcat_sb[:, k, :],
                wcat_t.rearrange("(t p) d -> t p d", p=P)[k],
            )
            nc.gpsimd.dma_start(
                xf8_tiles[0][:, k, :],
                x_f8[0].rearrange("(t p) f -> t p f", p=P)[k],
            )

        wfu_sb = wpool.tile([P, CT, C], F8)
        nc.gpsimd.dma_start(
            wfu_sb[:],
            wfu_t.rearrange("(t p) d -> p t d", p=P),
        )
        bcat_sb = wpool.tile([P, MT_D], F32)
        nc.sync.dma_start(
            bcat_sb.rearrange("p (t o) -> p t o", o=1),
            b_cat.rearrange("(t p) o -> p t o", p=P),
        )
        for s in range(NS):
            # ---- load x (fp8 for matmuls, bf16 residual w/ b_fuse folded) ----
            if s not in xf8_tiles:
                xf8_tiles[s] = xf8_pool.tile(
                    [P, CT, HW], F8, tag="xf8", name=f"xf8{s}"
                )
                nc.sync.dma_start(
                    xf8_tiles[s][:],
                    x_f8[s].rearrange("(t p) f -> p t f", p=P),
                )
            xf8_sb = xf8_tiles[s]
            xres_sb = xres_pool.tile([P, CT, HW], BF16, tag="xres")
            nc.sync.dma_start(
                xres_sb[:],
                x_res[s].rearrange("(t p) f -> p t f", p=P),
            )

            # ---- combined theta/phi 1x1 conv: tp = wcat.T @ x + b ----
            tp_sb = tp_pool.tile([P, MT_D, HW], F8, tag="tp")

            def conv_mm(ps, m, kp):
                for nq in range(NQ):
                    nc.tensor.matmul(
                        ps[:, nq * 512:(nq + 1) * 512],
                        wcat_sb[:, 2 * kp:2 * kp + 2, m * P:(m + 1) * P],
                        xf8_sb[:, 2 * kp:2 * kp + 2, nq * 512:nq * 512 + 512],
                        start=(kp == 0),
                        stop=(kp == CT // 2 - 1),
                        perf_mode=DR,
                    )

            def conv_copy(ps, m):
                # bias-add + fp8 convert, split in halves across DVE/ACT so
                # the last tp chunk lands before S(0) needs it (~3.6us)
                nc.vector.tensor_scalar_add(
                    tp_sb[:, m, 0:512], ps[:, 0:512], bcat_sb[:, m:m + 1],
                )
                nc.scalar.add(
                    tp_sb[:, m, 512:HW], ps[:, 512:HW], bcat_sb[:, m:m + 1],
                )

            if s == 0:
                # k-outer so each arriving x k-pair feeds all m immediately;
                # 4 live psums span both pools
                ps_cvs = [
                    (psum_y if m < 2 else psum_rot).tile(
                        [P, HW], F32, tag="mm", name=f"ps_cv0_{m}")
                    for m in range(MT_D)
                ]
                for kp in range(CT // 2):
                    for m in range(MT_D):
                        conv_mm(ps_cvs[m], m, kp)
                for m in range(MT_D):
                    conv_copy(ps_cvs[m], m)
            else:
                for m in range(MT_D):
                    ps_cv = psum_rot.tile(
                        [P, HW], F32, tag="mm", name=f"ps_cv{s}_{m}"
                    )
                    for kp in range(CT // 2):
                        conv_mm(ps_cv, m, kp)
                    conv_copy(ps_cv, m)

            gt_sb = gt_pool.tile([P, KT, C], F8, tag="gt")
            e_sb = e_pool.tile([P, KT, HW], F8, tag="e")

            def s_mtile(m):
                ps_s = psum_rot.tile([P, HW], F32, tag="mm", name=f"ps_s{s}_{m}")
                for nq in range(NQ):
                    nc.tensor.matmul(
                        ps_s[:, nq * 512:(nq + 1) * 512],
                        tp_sb[:, 2:4, m * P:(m + 1) * P],
                        tp_sb[:, 0:2, nq * 512:nq * 512 + 512],
                        start=True,
                        stop=True,
                        perf_mode=DR,
                    )
                nc.scalar.activation(
                    e_sb[:, m, :], ps_s[:], AF.Exp,
                    bias=ebias_sb[:], scale=float(EXP_SCALE),
                )

            def gt_pair(j):
                # two G'T m-tiles share one psum; single DVE copy (DVE is
                # otherwise idle during the kp loop)
                ps_g = psum_rot.tile([P, HW], F32, tag="mm", name=f"ps_g{s}_{j}")
                for mi in range(2):
                    m = 2 * j + mi
                    for kp in range(CT // 2):
                        nc.tensor.matmul(
                            ps_g[:, mi * C:(mi + 1) * C],
                            xf8_sb[:, 2 * kp:2 * kp + 2, m * P:(m + 1) * P],
                            wfu_sb[:, 2 * kp:2 * kp + 2, :],
                            start=(kp == 0),
                            stop=(kp == CT // 2 - 1),
                            perf_mode=DR,
                        )
                nc.vector.tensor_copy(gt_sb[:, 2 * j:2 * j + 2, :], ps_g[:])

            def y_kp(ps_y, m, kp):
                for nq in range(NQ):
                    nc.tensor.matmul(
                        ps_y[:, nq * 512:(nq + 1) * 512],
                        gt_sb[:, 2 * kp:2 * kp + 2, m * P:(m + 1) * P],
                        e_sb[:, 2 * kp:2 * kp + 2, nq * 512:nq * 512 + 512],
                        start=(kp == 0),
                        stop=(kp == KP - 1),
                        perf_mode=DR,
                        skip_group_check=True,
                    )

            def z_kp(ps_z, kp):
                for nq in range(NQ):
                    nc.tensor.matmul(
                        ps_z[:, nq * 512:(nq + 1) * 512],
                        ones_f8[:],
                        e_sb[:, 2 * kp:2 * kp + 2, nq * 512:nq * 512 + 512],
                        start=(kp == 0),
                        stop=(kp == KP - 1),
                        perf_mode=DR,
                        skip_group_check=True,
                    )

            if s == 0:
                # wfu rides the gpsimd ring behind the x chunks (~10us): run
                # all of S first for cover, then gt, then y/Z
                for m in range(KT):
                    s_mtile(m)
                for j in range(KT // 2):
                    gt_pair(j)
                ps_y0 = psum_y.tile([P, HW], F32, tag="mm", name=f"ps_y{s}_0")
                ps_y1 = psum_y.tile([P, HW], F32, tag="mm", name=f"ps_y{s}_1")
                ps_z = psum_rot.tile([P, HW], F32, tag="mm", name=f"ps_z{s}")
                for kp in range(KP):
                    y_kp(ps_y0, 0, kp)
                    y_kp(ps_y1, 1, kp)
                    z_kp(ps_z, kp)
            else:
                # steady state: S feeds ACT; gt/y0/y1/Z matmuls fill the PE
                # while ACT chews exps; y0/y1 accumulate in resident psum
                ps_y0 = psum_y.tile([P, HW], F32, tag="mm", name=f"ps_y{s}_0")
                ps_y1 = psum_y.tile([P, HW], F32, tag="mm", name=f"ps_y{s}_1")
                ps_z = None
                for kp in range(KP):
                    s_mtile(2 * kp)
                    s_mtile(2 * kp + 1)
                    gt_pair(kp)
                    y_kp(ps_y0, 0, kp)
                    y_kp(ps_y1, 1, kp)
                    if kp == 0:
                        ps_z = psum_rot.tile(
                            [P, HW], F32, tag="mm", name=f"ps_z{s}"
                        )
                    z_kp(ps_z, kp)

            rzb_sb = rz_pool.tile([P, HW], F32, tag="rz")
            nc.vector.reciprocal_approx_fast(out=rzb_sb[:, 0:512], in_=ps_z[:, 0:512])
            nc.vector.reciprocal_approx_fast(out=rzb_sb[:, 512:HW], in_=ps_z[:, 512:HW])

            def y_mtile(m, pool):
                ps_y = pool.tile([P, HW], F32, tag="mm", name=f"ps_y{s}_{m}")
                for kp in range(KP):
                    y_kp(ps_y, m, kp)
                return ps_y

            ps_ys = [ps_y0, ps_y1, y_mtile(2, psum_rot), y_mtile(3, psum_y)]

            # final combine: mul on DVE (psum src), add in bf16 (2x DVE mode);
            # the last sample's adds go to GpSimd (idle at the kernel tail)
            for m in range(CT):
                t1 = fin_pool.tile([P, HW], BF16, tag="fin", name=f"t1_{s}_{m}")
                o_sb = out_pool.tile([P, HW], BF16, tag="o", name=f"o_{s}_{m}")
                add_eng = nc.gpsimd if s == NS - 1 else nc.vector
                nc.vector.tensor_mul(t1[:], ps_ys[m][:], rzb_sb[:])
                add_eng.tensor_add(o_sb[:], t1[:], xres_sb[:, m, :])
                nc.sync.dma_start(
                    out_d[s].rearrange("(t p) f -> t p f", p=P)[m],
                    o_sb[:],
                )


_CACHE = {}


def _build():
    if "nc" not in _CACHE:
        nc = bacc.Bacc("TRN2", target_bir_lowering=False, debug=False)
        with tile.TileContext(nc) as tc:
            _emit(tc)
        nc.compile()
        _CACHE["nc"] = nc
    return _CACHE["nc"]


def _prep_in_maps(x, W_theta, b_theta, W_phi, b_phi, W_fuse, b_fuse):
    bf = ml_dtypes.bfloat16
    f8 = ml_dtypes.float8_e4m3
    xf = np.ascontiguousarray(x.reshape(N, C, HW).astype(np.float32))
    x_f8 = xf.astype(f8)
    x_res = (xf + b_fuse.astype(np.float32)[None, :, None]).astype(bf)
    wcat_t = np.ascontiguousarray(
        np.concatenate([W_theta.astype(np.float32) * TP_BOOST,
                        W_phi.astype(np.float32) * TP_BOOST], axis=0).T
    ).astype(f8)
    b_cat = np.concatenate([b_theta.astype(np.float32) * TP_BOOST,
                            b_phi.astype(np.float32) * TP_BOOST]).reshape(2 * D, 1)
    wfu_t = np.ascontiguousarray(
        W_fuse.astype(np.float32).T * FU_BOOST
    ).astype(f8)

    in_maps = []
    for c in range(NCORES):
        sl = slice(c * NS, (c + 1) * NS)
        in_maps.append({
            "x_f8": np.ascontiguousarray(x_f8[sl]),
            "x_res": np.ascontiguousarray(x_res[sl]),
            "wcat_t": wcat_t,
            "b_cat": b_cat.astype(np.float32),
            "wfu_t": wfu_t,
        })
    return in_maps


def _run(inputs, trace=False, **kw):
    nc = _build()
    in_maps = _prep_in_maps(**inputs)
    res = bass_utils.run_bass_kernel_spmd(
        nc, in_maps, core_ids=list(range(NCORES)), trace=trace, **kw
    )
    out = np.concatenate(
        [res.results[c]["out"].astype(np.float32) for c in range(NCORES)], axis=0
    )
    return out.reshape(N, C, H, W), res


def kernel(**inputs):
    inputs = {k: np.asarray(v) for k, v in inputs.items()}
    out, _ = _run(inputs, trace=False)
    return out


# revision 10
# speedup vs baseline: 1.3547x; 1.3547x over previous
"""Non-local (spatial self-attention) denoising block on 8 Trainium2 cores.

Reference math (per sample n, with x:[C,HW], D=C/2):
    t = (W_theta @ x + b_theta) / sqrt(D)      [D, HW]
    p = W_phi   @ x + b_phi                    [D, HW]
    S[q,k] = t[:,q] . p[:,k]
    f = softmax_k(S)
    attn = x @ f.T
    out = x + W_fuse @ attn + b_fuse

Device formulation -- all matmul phases run fp8e4 (e4m3) DoubleRow, which
packs two 128-row k-tiles per instruction (~1.9x bf16 PE throughput):
    Sᵀ[k,q] = p.T @ t    -- keys on partitions so no transposes are needed
    e = exp(Sᵀ/1024 - 3) -- shift is softmax-invariant; keeps e in e4m3 range
    G'ᵀ = xᵀ @ (16·W_fuseᵀ)  -- output conv fused into the values: [HW, C]
    Zb = (16·ones)ᵀ @ e  -- Z on all 128 partitions via accumulating DR
                            matmuls over the 8 e tiles (replaces a 7-op DVE
                            f32 add chain that was the v1 bottleneck)
    y = G'ᵀᵀ @ e          -- [C, HW] unnormalized
    out = y * (1/Zb) + (x + b_fuse)

fp8 scale management (e4m3: max 240, min normal 2^-7): W_theta/W_phi are
boosted x8 host-side (raw 0.05-scale weights would sit in subnormals),
W_fuse x16; the 1/sqrt(D) softmax scale and both theta/phi boosts fold into
the exp activation's scale (1/1024), the W_fuse boost into the Z ones.

Schedule (per sample, steady state): conv -> kp-loop{S(2kp), S(2kp+1),
gt-pair(kp), y0(kp), y1(kp), Z(kp)} -> y2, y3 -> combine. y0/y1 accumulate
in a RESIDENT psum pool across the kp loop so the y matmuls consume e tiles
as ACT produces them -- no exp-lag stall before the y phase. PSUM: resident
pool (y0,y1,y3) 4 banks + rotating pool (conv/S/gt/Z/y2) 4 banks.
Engine balance per sample: PE ~21us, ACT ~11.5us (exps + half the conv
copies), DVE ~16us (other copies, gt copies, recip, final mul/add).

Sharding: data-parallel over batch N=32 -> 4 samples per core on 8 cores.
A ~3.8us burst of junk matmuls pre-warms the PE clock (HAM) while the first
DMAs land; a dummy exp pre-loads the ACT spline table.
"""

import numpy as np
import ml_dtypes

import concourse.bass as bass
import concourse.tile as tile
from concourse import bacc, mybir
from concourse import bass_utils

F32 = mybir.dt.float32
BF16 = mybir.dt.bfloat16
F8 = mybir.dt.float8e4
AF = mybir.ActivationFunctionType
DR = mybir.MatmulPerfMode.DoubleRow

N, C, H, W = 32, 512, 32, 32
D = C // 2
HW = H * W
NCORES = 8
NS = N // NCORES  # samples per core
P = 128
CT = C // P   # 4 c-tiles
KT = HW // P  # 8 hw-tiles
MT_D = (2 * D) // P  # 4 m-tiles of combined theta/phi conv
NQ = HW // 512  # 2 free-dim halves
KP = KT // 2  # 4 k-pairs for DoubleRow over HW

TP_BOOST = 8.0     # on W_theta and W_phi (and their biases)
FU_BOOST = 16.0    # on W_fuse
EXP_SCALE = 1.0 / (TP_BOOST * TP_BOOST * np.sqrt(np.float32(D)))
EXP_BIAS = -3.0    # softmax-shift: keeps exp() within e4m3 range


def _emit(tc):
    nc = tc.nc

    x_f8 = nc.dram_tensor("x_f8", [NS, C, HW], F8, kind="ExternalInput").ap()
    x_res = nc.dram_tensor("x_res", [NS, C, HW], BF16, kind="ExternalInput").ap()
    wcat_t = nc.dram_tensor("wcat_t", [C, 2 * D], F8, kind="ExternalInput").ap()
    b_cat = nc.dram_tensor("b_cat", [2 * D, 1], F32, kind="ExternalInput").ap()
    wfu_t = nc.dram_tensor("wfu_t", [C, C], F8, kind="ExternalInput").ap()
    out_d = nc.dram_tensor("out", [NS, C, HW], BF16, kind="ExternalOutput").ap()

    import contextlib
    ctx = contextlib.ExitStack()
    with ctx:
        # ---- constant pools ----
        wpool = ctx.enter_context(tc.tile_pool(name="wpool", bufs=1))
        wcat_sb = wpool.tile([P, CT, 2 * D], F8)

        # ---- working pools ----
        xf8_pool = ctx.enter_context(tc.tile_pool(name="xf8", bufs=3))
        xres_pool = ctx.enter_context(tc.tile_pool(name="xres", bufs=2))
        tp_pool = ctx.enter_context(tc.tile_pool(name="tp", bufs=2))
        gt_pool = ctx.enter_context(tc.tile_pool(name="gt", bufs=2))
        e_pool = ctx.enter_context(tc.tile_pool(name="e", bufs=2))
        rz_pool = ctx.enter_context(tc.tile_pool(name="rz", bufs=2))
        fin_pool = ctx.enter_context(tc.tile_pool(name="fin", bufs=3))
        out_pool = ctx.enter_context(tc.tile_pool(name="outp", bufs=3))

        # PSUM: 1-buf resident pool for the kp-loop-accumulating y0 (+y3),
        # 3-buf rotating pool for everything else. 2 + 3x2 banks = all 8.
        psum_y = ctx.enter_context(tc.tile_pool(name="psy", bufs=1, space="PSUM"))
        psum_rot = ctx.enter_context(tc.tile_pool(name="psr", bufs=3, space="PSUM"))

        # HAM pre-warm + ACT exp-table pre-load during the initial DMA wait
        ones_f8 = wpool.tile([P, 2, P], F8)
        nc.vector.memset(ones_f8[:], FU_BOOST)  # folds the wfu boost out of Z
        warm_rhs = wpool.tile([P, 512], BF16)
        nc.vector.memset(warm_rhs[:], 0.0)
        ebias_sb = wpool.tile([P, 1], F32)
        nc.vector.memset(ebias_sb[:], EXP_BIAS)
        warm_e = wpool.tile([P, 16], F8)
        nc.scalar.activation(warm_e[:], warm_rhs[:, 0:16], AF.Exp,
                             bias=ebias_sb[:], scale=float(EXP_SCALE))
        ps_warm = psum_rot.tile([P, 512], F32, tag="mm", name="ps_warm")
        warm_lhs = wpool.tile([P, P], BF16)
        nc.vector.memset(warm_lhs[:], 0.0)
        for w in range(9):
            nc.tensor.matmul(ps_warm[:], warm_lhs[:], warm_rhs[:],
                             start=True, stop=True)

        xf8_tiles = {}
        xf8_tiles[0] = xf8_pool.tile([P, CT, HW], F8, tag="xf8", name="xf80")
        for k in range(CT):
            nc.sync.dma_start(
                wcat_sb[:, k, :],
                wcat_t.rearrange("(t p) d -> t p d", p=P)[k],
            )
            nc.gpsimd.dma_start(
                xf8_tiles[0][:, k, :],
                x_f8[0].rearrange("(t p) f -> t p f", p=P)[k],
            )

        wfu_sb = wpool.tile([P, CT, C], F8)
        nc.gpsimd.dma_start(
            wfu_sb[:],
            wfu_t.rearrange("(t p) d -> p t d", p=P),
        )
        bcat_sb = wpool.tile([P, MT_D], F32)
        nc.sync.dma_start(
            bcat_sb.rearrange("p (t o) -> p t o", o=1),
            b_cat.rearrange("(t p) o -> p t o", p=P),
        )
        for s in range(NS):
            # ---- load x (fp8 for matmuls, bf16 residual w/ b_fuse folded) ----
            if s not in xf8_tiles:
                xf8_tiles[s] = xf8_pool.tile(
                    [P, CT, HW], F8, tag="xf8", name=f"xf8{s}"
                )
                nc.sync.dma_start(
                    xf8_tiles[s][:],
                    x_f8[s].rearrange("(t p) f -> p t f", p=P),
                )
            xf8_sb = xf8_tiles[s]
            xres_sb = xres_pool.tile([P, CT, HW], BF16, tag="xres")
            nc.sync.dma_start(
                xres_sb[:],
                x_res[s].rearrange("(t p) f -> p t f", p=P),
            )

            # ---- combined theta/phi 1x1 conv: tp = wcat.T @ x + b ----
            tp_sb = tp_pool.tile([P, MT_D, HW], F8, tag="tp")

            def conv_mm(ps, m, kp):
                for nq in range(NQ):
                    nc.tensor.matmul(
                        ps[:, nq * 512:(nq + 1) * 512],
                        wcat_sb[:, 2 * kp:2 * kp + 2, m * P:(m + 1) * P],
                        xf8_sb[:, 2 * kp:2 * kp + 2, nq * 512:nq * 512 + 512],
                        start=(kp == 0),
                        stop=(kp == CT // 2 - 1),
                        perf_mode=DR,
                    )

            def conv_copy(ps, m):
                # bias-add + fp8 convert, split in halves across DVE/ACT so
                # the last tp chunk lands before S(0) needs it (~3.6us)
                nc.vector.tensor_scalar_add(
                    tp_sb[:, m, 0:512], ps[:, 0:512], bcat_sb[:, m:m + 1],
                )
                nc.scalar.add(
                    tp_sb[:, m, 512:HW], ps[:, 512:HW], bcat_sb[:, m:m + 1],
                )

            if s == 0:
                # k-outer so each arriving x k-pair feeds all m immediately;
                # 4 live psums span both pools
                ps_cvs = [
                    (psum_y if m < 1 else psum_rot).tile(
                        [P, HW], F32, tag="mm", name=f"ps_cv0_{m}")
                    for m in range(MT_D)
                ]
                for kp in range(CT // 2):
                    for m in range(MT_D):
                        conv_mm(ps_cvs[m], m, kp)
                for m in range(MT_D):
                    conv_copy(ps_cvs[m], m)
            else:
                for m in range(MT_D):
                    ps_cv = psum_rot.tile(
                        [P, HW], F32, tag="mm", name=f"ps_cv{s}_{m}"
                    )
                    for kp in range(CT // 2):
                        conv_mm(ps_cv, m, kp)
                    conv_copy(ps_cv, m)

            gt_sb = gt_pool.tile([P, KT, C], F8, tag="gt")
            e_sb = e_pool.tile([P, KT, HW], F8, tag="e")

            def s_mtile(m):
                ps_s = psum_rot.tile([P, HW], F32, tag="mm", name=f"ps_s{s}_{m}")
                for nq in range(NQ):
                    nc.tensor.matmul(
                        ps_s[:, nq * 512:(nq + 1) * 512],
                        tp_sb[:, 2:4, m * P:(m + 1) * P],
                        tp_sb[:, 0:2, nq * 512:nq * 512 + 512],
                        start=True,
                        stop=True,
                        perf_mode=DR,
                    )
                nc.scalar.activation(
                    e_sb[:, m, :], ps_s[:], AF.Exp,
                    bias=ebias_sb[:], scale=float(EXP_SCALE),
                )

            def gt_pair(j):
                # two G'T m-tiles share one psum; single DVE copy (DVE is
                # otherwise idle during the kp loop)
                ps_g = psum_rot.tile([P, HW], F32, tag="mm", name=f"ps_g{s}_{j}")
                for mi in range(2):
                    m = 2 * j + mi
                    for kp in range(CT // 2):
                        nc.tensor.matmul(
                            ps_g[:, mi * C:(mi + 1) * C],
                            xf8_sb[:, 2 * kp:2 * kp + 2, m * P:(m + 1) * P],
                            wfu_sb[:, 2 * kp:2 * kp + 2, :],
                            start=(kp == 0),
                            stop=(kp == CT // 2 - 1),
                            perf_mode=DR,
                        )
                nc.vector.tensor_copy(gt_sb[:, 2 * j:2 * j + 2, :], ps_g[:])

            def y_kp(ps_y, m, kp):
                for nq in range(NQ):
                    nc.tensor.matmul(
                        ps_y[:, nq * 512:(nq + 1) * 512],
                        gt_sb[:, 2 * kp:2 * kp + 2, m * P:(m + 1) * P],
                        e_sb[:, 2 * kp:2 * kp + 2, nq * 512:nq * 512 + 512],
                        start=(kp == 0),
                        stop=(kp == KP - 1),
                        perf_mode=DR,
                        skip_group_check=True,
                    )

            def z_kp(ps_z, kp):
                for nq in range(NQ):
                    nc.tensor.matmul(
                        ps_z[:, nq * 512:(nq + 1) * 512],
                        ones_f8[:],
                        e_sb[:, 2 * kp:2 * kp + 2, nq * 512:nq * 512 + 512],
                        start=(kp == 0),
                        stop=(kp == KP - 1),
                        perf_mode=DR,
                        skip_group_check=True,
                    )

            ps_y0 = psum_y.tile([P, HW], F32, tag="mm", name=f"ps_y{s}_0")
            if s == 0:
                # wfu rides the gpsimd ring behind the x chunks (~10us): run
                # all of S first for cover, then gt, then y0/Z
                for m in range(KT):
                    s_mtile(m)
                for j in range(KT // 2):
                    gt_pair(j)
                for kp in range(KP):
                    y_kp(ps_y0, 0, kp)
            else:
                # steady state: S feeds ACT; gt fills PE while ACT chews exps;
                # y0 consumes the PREVIOUS kp's e/gt tiles (one-step software
                # pipeline) so it never waits on this kp's exp or gt copy
                for kp in range(KP):
                    s_mtile(2 * kp)
                    s_mtile(2 * kp + 1)
                    gt_pair(kp)
                    if kp > 0:
                        y_kp(ps_y0, 0, kp - 1)
                y_kp(ps_y0, 0, KP - 1)

            # Z right after the loop: its last k-pair is gated by the final
            # exp, which ACT delivers just as the PE drains the kp loop
            ps_z = psum_rot.tile([P, HW], F32, tag="mm", name=f"ps_z{s}")
            for kp in range(KP):
                z_kp(ps_z, kp)

            rzb_sb = rz_pool.tile([P, HW], F32, tag="rz")
            nc.vector.reciprocal_approx_fast(out=rzb_sb[:, 0:512], in_=ps_z[:, 0:512])
            nc.vector.reciprocal_approx_fast(out=rzb_sb[:, 512:HW], in_=ps_z[:, 512:HW])

            def combine(m, ps_y):
                # mul on DVE (psum src); add in bf16 (2x DVE mode); emitted
                # right after y(m) so it pipelines behind y(m+1)'s matmuls
                t1 = fin_pool.tile([P, HW], BF16, tag="fin", name=f"t1_{s}_{m}")
                o_sb = out_pool.tile([P, HW], BF16, tag="o", name=f"o_{s}_{m}")
                nc.vector.tensor_mul(t1[:], ps_y[:], rzb_sb[:])
                nc.vector.tensor_add(o_sb[:], t1[:], xres_sb[:, m, :])
                nc.sync.dma_start(
                    out_d[s].rearrange("(t p) f -> t p f", p=P)[m],
                    o_sb[:],
                )

            prev = (0, ps_y0)
            for m in range(1, CT):
                pool = psum_y if m == CT - 1 else psum_rot
                ps_y = pool.tile([P, HW], F32, tag="mm", name=f"ps_y{s}_{m}")
                for kp in range(KP):
                    y_kp(ps_y, m, kp)
                combine(*prev)
                prev = (m, ps_y)
            combine(*prev)


_CACHE = {}


def _build():
    if "nc" not in _CACHE:
        nc = bacc.Bacc("TRN2", target_bir_lowering=False, debug=False)
        with tile.TileContext(nc) as tc:
            _emit(tc)
        nc.compile()
        _CACHE["nc"] = nc
    return _CACHE["nc"]


def _prep_in_maps(x, W_theta, b_theta, W_phi, b_phi, W_fuse, b_fuse):
    bf = ml_dtypes.bfloat16
    f8 = ml_dtypes.float8_e4m3
    xf = np.ascontiguousarray(x.reshape(N, C, HW).astype(np.float32))
    x_f8 = xf.astype(f8)
    x_res = (xf + b_fuse.astype(np.float32)[None, :, None]).astype(bf)
    wcat_t = np.ascontiguousarray(
        np.concatenate([W_theta.astype(np.float32) * TP_BOOST,
                        W_phi.astype(np.float32) * TP_BOOST], axis=0).T
    ).astype(f8)
    b_cat = np.concatenate([b_theta.astype(np.float32) * TP_BOOST,
                            b_phi.astype(np.float32) * TP_BOOST]).reshape(2 * D, 1)
    wfu_t = np.ascontiguousarray(
        W_fuse.astype(np.float32).T * FU_BOOST
    ).astype(f8)

    in_maps = []
    for c in range(NCORES):
        sl = slice(c * NS, (c + 1) * NS)
        in_maps.append({
            "x_f8": np.ascontiguousarray(x_f8[sl]),
            "x_res": np.ascontiguousarray(x_res[sl]),
            "wcat_t": wcat_t,
            "b_cat": b_cat.astype(np.float32),
            "wfu_t": wfu_t,
        })
    return in_maps


def _run(inputs, trace=False, **kw):
    nc = _build()
    in_maps = _prep_in_maps(**inputs)
    res = bass_utils.run_bass_kernel_spmd(
        nc, in_maps, core_ids=list(range(NCORES)), trace=trace, **kw
    )
    out = np.concatenate(
        [res.results[c]["out"].astype(np.float32) for c in range(NCORES)], axis=0
    )
    return out.reshape(N, C, H, W), res


def kernel(**inputs):
    inputs = {k: np.asarray(v) for k, v in inputs.items()}
    out, _ = _run(inputs, trace=False)
    return out


# revision 17
# speedup vs baseline: 1.5079x; 1.1131x over previous
"""Non-local (spatial self-attention) denoising block on 8 Trainium2 cores.

Reference math (per sample n, with x:[C,HW], D=C/2):
    t = (W_theta @ x + b_theta) / sqrt(D)      [D, HW]
    p = W_phi   @ x + b_phi                    [D, HW]
    S[q,k] = t[:,q] . p[:,k]
    f = softmax_k(S)
    attn = x @ f.T
    out = x + W_fuse @ attn + b_fuse

Device formulation -- all matmul phases run fp8e4 (e4m3) DoubleRow, which
packs two 128-row k-tiles per instruction (~1.9x bf16 PE throughput):
    Sᵀ[k,q] = p.T @ t    -- keys on partitions so no transposes are needed
    e = exp(Sᵀ/1024 - 3) -- shift is softmax-invariant; keeps e in e4m3 range
    G'ᵀ = xᵀ @ (16·W_fuseᵀ)  -- output conv fused into the values: [HW, C]
    Zb = (16·ones)ᵀ @ e  -- Z on all 128 partitions via accumulating DR
                            matmuls over the 8 e tiles (replaces a 7-op DVE
                            f32 add chain that was the v1 bottleneck)
    y = G'ᵀᵀ @ e          -- [C, HW] unnormalized
    out = y * (1/Zb) + (x + b_fuse)

fp8 scale management (e4m3: max 240, min normal 2^-7): W_theta/W_phi are
boosted x8 host-side (raw 0.05-scale weights would sit in subnormals),
W_fuse x16; the 1/sqrt(D) softmax scale and both theta/phi boosts fold into
the exp activation's scale (1/1024), the W_fuse boost into the Z ones.

Schedule (per sample, steady state): conv -> kp-loop{S(2kp), S(2kp+1),
gt-pair(kp), y0(kp), y1(kp), Z(kp)} -> y2, y3 -> combine. y0/y1 accumulate
in a RESIDENT psum pool across the kp loop so the y matmuls consume e tiles
as ACT produces them -- no exp-lag stall before the y phase. PSUM: resident
pool (y0,y1,y3) 4 banks + rotating pool (conv/S/gt/Z/y2) 4 banks.
Engine balance per sample: PE ~21us, ACT ~11.5us (exps + half the conv
copies), DVE ~16us (other copies, gt copies, recip, final mul/add).

Sharding: data-parallel over batch N=32 -> 4 samples per core on 8 cores.
A ~3.8us burst of junk matmuls pre-warms the PE clock (HAM) while the first
DMAs land; a dummy exp pre-loads the ACT spline table.
"""

import numpy as np
import ml_dtypes

import concourse.bass as bass
import concourse.tile as tile
from concourse import bacc, mybir
from concourse import bass_utils

F32 = mybir.dt.float32
BF16 = mybir.dt.bfloat16
F8 = mybir.dt.float8e4
AF = mybir.ActivationFunctionType
DR = mybir.MatmulPerfMode.DoubleRow

N, C, H, W = 32, 512, 32, 32
D = C // 2
HW = H * W
NCORES = 8
NS = N // NCORES  # samples per core
P = 128
CT = C // P   # 4 c-tiles
KT = HW // P  # 8 hw-tiles
MT_D = (2 * D) // P  # 4 m-tiles of combined theta/phi conv
NQ = HW // 512  # 2 free-dim halves
KP = KT // 2  # 4 k-pairs for DoubleRow over HW

TP_BOOST = 8.0     # on W_theta and W_phi (and their biases)
FU_BOOST = 16.0    # on W_fuse
EXP_SCALE = 1.0 / (TP_BOOST * TP_BOOST * np.sqrt(np.float32(D)))
EXP_BIAS = -3.0    # softmax-shift: keeps exp() within e4m3 range


def _emit(tc):
    nc = tc.nc

    x_f8 = nc.dram_tensor("x_f8", [NS, C, HW], F8, kind="ExternalInput").ap()
    x_res = nc.dram_tensor("x_res", [NS, C, HW], BF16, kind="ExternalInput").ap()
    wcat_t = nc.dram_tensor("wcat_t", [C, 2 * D], F8, kind="ExternalInput").ap()
    b_cat = nc.dram_tensor("b_cat", [2 * D, 1], F32, kind="ExternalInput").ap()
    wfu_t = nc.dram_tensor("wfu_t", [C, C], F8, kind="ExternalInput").ap()
    out_d = nc.dram_tensor("out", [NS, C, HW], BF16, kind="ExternalOutput").ap()

    import contextlib
    ctx = contextlib.ExitStack()
    with ctx:
        # ---- constant pools ----
        wpool = ctx.enter_context(tc.tile_pool(name="wpool", bufs=1))
        wcat_sb = wpool.tile([P, CT, 2 * D], F8)

        # ---- working pools ----
        xf8_pool = ctx.enter_context(tc.tile_pool(name="xf8", bufs=3))
        xres_pool = ctx.enter_context(tc.tile_pool(name="xres", bufs=2))
        tp_pool = ctx.enter_context(tc.tile_pool(name="tp", bufs=2))
        gt_pool = ctx.enter_context(tc.tile_pool(name="gt", bufs=2))
        e_pool = ctx.enter_context(tc.tile_pool(name="e", bufs=2))
        rz_pool = ctx.enter_context(tc.tile_pool(name="rz", bufs=2))
        fin_pool = ctx.enter_context(tc.tile_pool(name="fin", bufs=3))
        out_pool = ctx.enter_context(tc.tile_pool(name="outp", bufs=3))

        # PSUM: 1-buf resident pool for the kp-loop-accumulating y0 (+y3),
        # 3-buf rotating pool for everything else. 2 + 3x2 banks = all 8.
        psum_y = ctx.enter_context(tc.tile_pool(name="psy", bufs=1, space="PSUM"))
        psum_rot = ctx.enter_context(tc.tile_pool(name="psr", bufs=3, space="PSUM"))

        # HAM pre-warm + ACT exp-table pre-load during the initial DMA wait
        ones_f8 = wpool.tile([P, 2, P], F8)
        nc.vector.memset(ones_f8[:], FU_BOOST)  # folds the wfu boost out of Z
        warm_rhs = wpool.tile([P, 512], BF16)
        nc.vector.memset(warm_rhs[:], 0.0)
        ebias_sb = wpool.tile([P, 1], F32)
        nc.vector.memset(ebias_sb[:], EXP_BIAS)
        warm_e = wpool.tile([P, 16], F8)
        nc.scalar.activation(warm_e[:], warm_rhs[:, 0:16], AF.Exp,
                             bias=ebias_sb[:], scale=float(EXP_SCALE))
        ps_warm = psum_rot.tile([P, 512], F32, tag="mm", name="ps_warm")
        warm_lhs = wpool.tile([P, P], BF16)
        nc.vector.memset(warm_lhs[:], 0.0)
        for w in range(9):
            nc.tensor.matmul(ps_warm[:], warm_lhs[:], warm_rhs[:],
                             start=True, stop=True)
        # read ps_warm so walrus can't dead-code-eliminate the HAM warmup
        warm_sink = wpool.tile([P, 8], F32)
        nc.vector.tensor_copy(warm_sink[:], ps_warm[:, 0:8])

        xf8_tiles = {}
        xf8_tiles[0] = xf8_pool.tile([P, CT, HW], F8, tag="xf8", name="xf80")
        for k in range(CT):
            nc.sync.dma_start(
                wcat_sb[:, k, :],
                wcat_t.rearrange("(t p) d -> t p d", p=P)[k],
            )
            nc.gpsimd.dma_start(
                xf8_tiles[0][:, k, :],
                x_f8[0].rearrange("(t p) f -> t p f", p=P)[k],
            )

        # bcat (tiny, needed by the first conv copies) first, then wfu on the
        # sync ring right behind wcat so sample 0's gt matmuls don't wait
        bcat_sb = wpool.tile([P, MT_D], F32)
        nc.sync.dma_start(
            bcat_sb.rearrange("p (t o) -> p t o", o=1),
            b_cat.rearrange("(t p) o -> p t o", p=P),
        )
        wfu_sb = wpool.tile([P, CT, C], F8)
        nc.sync.dma_start(
            wfu_sb[:],
            wfu_t.rearrange("(t p) d -> p t d", p=P),
        )
        for s in range(NS):
            # ---- load x (fp8 for matmuls, bf16 residual w/ b_fuse folded) ----
            if s not in xf8_tiles:
                xf8_tiles[s] = xf8_pool.tile(
                    [P, CT, HW], F8, tag="xf8", name=f"xf8{s}"
                )
                nc.sync.dma_start(
                    xf8_tiles[s][:],
                    x_f8[s].rearrange("(t p) f -> p t f", p=P),
                )
            xf8_sb = xf8_tiles[s]
            xres_sb = xres_pool.tile([P, CT, HW], BF16, tag="xres")
            nc.sync.dma_start(
                xres_sb[:],
                x_res[s].rearrange("(t p) f -> p t f", p=P),
            )

            # ---- combined theta/phi 1x1 conv: tp = wcat.T @ x + b ----
            tp_sb = tp_pool.tile([P, MT_D, HW], F8, tag="tp")

            def conv_mm(ps, m, kp):
                for nq in range(NQ):
                    nc.tensor.matmul(
                        ps[:, nq * 512:(nq + 1) * 512],
                        wcat_sb[:, 2 * kp:2 * kp + 2, m * P:(m + 1) * P],
                        xf8_sb[:, 2 * kp:2 * kp + 2, nq * 512:nq * 512 + 512],
                        start=(kp == 0),
                        stop=(kp == CT // 2 - 1),
                        perf_mode=DR,
                    )

            def conv_copy(ps, m):
                # bias-add + fp8 convert on ACT (Identity w/ per-partition
                # bias): ACT is idle at conv time, while DVE is still
                # draining the previous sample's combine ops
                nc.scalar.add(tp_sb[:, m, :], ps[:], bcat_sb[:, m:m + 1])

            if s == 0:
                # k-outer so each arriving x k-pair feeds all m immediately;
                # 4 live psums span both pools
                ps_cvs = [
                    (psum_y if m < 1 else psum_rot).tile(
                        [P, HW], F32, tag="mm", name=f"ps_cv0_{m}")
                    for m in range(MT_D)
                ]
                for kp in range(CT // 2):
                    for m in range(MT_D):
                        conv_mm(ps_cvs[m], m, kp)
                for m in range(MT_D):
                    conv_copy(ps_cvs[m], m)
            else:
                for m in range(MT_D):
                    ps_cv = psum_rot.tile(
                        [P, HW], F32, tag="mm", name=f"ps_cv{s}_{m}"
                    )
                    for kp in range(CT // 2):
                        conv_mm(ps_cv, m, kp)
                    conv_copy(ps_cv, m)

            gt_sb = gt_pool.tile([P, KT, C], F8, tag="gt")
            e_sb = e_pool.tile([P, KT, HW], F8, tag="e")

            def s_mtile(m):
                ps_s = psum_rot.tile([P, HW], F32, tag="mm", name=f"ps_s{s}_{m}")
                for nq in range(NQ):
                    nc.tensor.matmul(
                        ps_s[:, nq * 512:(nq + 1) * 512],
                        tp_sb[:, 2:4, m * P:(m + 1) * P],
                        tp_sb[:, 0:2, nq * 512:nq * 512 + 512],
                        start=True,
                        stop=True,
                        perf_mode=DR,
                    )
                nc.scalar.activation(
                    e_sb[:, m, :], ps_s[:], AF.Exp,
                    bias=ebias_sb[:], scale=float(EXP_SCALE),
                )

            def gt_pair(j):
                # two G'T m-tiles share one psum; single DVE copy (DVE is
                # otherwise idle during the kp loop)
                ps_g = psum_rot.tile([P, HW], F32, tag="mm", name=f"ps_g{s}_{j}")
                for mi in range(2):
                    m = 2 * j + mi
                    for kp in range(CT // 2):
                        nc.tensor.matmul(
                            ps_g[:, mi * C:(mi + 1) * C],
                            xf8_sb[:, 2 * kp:2 * kp + 2, m * P:(m + 1) * P],
                            wfu_sb[:, 2 * kp:2 * kp + 2, :],
                            start=(kp == 0),
                            stop=(kp == CT // 2 - 1),
                            perf_mode=DR,
                        )
                nc.vector.tensor_copy(gt_sb[:, 2 * j:2 * j + 2, :], ps_g[:])

            def y_kp(ps_y, m, kp):
                for nq in range(NQ):
                    nc.tensor.matmul(
                        ps_y[:, nq * 512:(nq + 1) * 512],
                        gt_sb[:, 2 * kp:2 * kp + 2, m * P:(m + 1) * P],
                        e_sb[:, 2 * kp:2 * kp + 2, nq * 512:nq * 512 + 512],
                        start=(kp == 0),
                        stop=(kp == KP - 1),
                        perf_mode=DR,
                        skip_group_check=True,
                    )

            def z_kp(ps_z, kp):
                for nq in range(NQ):
                    nc.tensor.matmul(
                        ps_z[:, nq * 512:(nq + 1) * 512],
                        ones_f8[:],
                        e_sb[:, 2 * kp:2 * kp + 2, nq * 512:nq * 512 + 512],
                        start=(kp == 0),
                        stop=(kp == KP - 1),
                        perf_mode=DR,
                        skip_group_check=True,
                    )

            # kp-loop: gt FIRST in each slot (its psum-slot WAR wait and its
            # DVE CAST then get a full kp of slack before y0 consumes them);
            # y0 consumes the PREVIOUS kp's e/gt tiles (one-step software
            # pipeline) so it never waits on this kp's exp or gt copy
            ps_y0 = psum_y.tile([P, HW], F32, tag="mm", name=f"ps_y{s}_0")
            for kp in range(KP):
                gt_pair(kp)
                s_mtile(2 * kp)
                s_mtile(2 * kp + 1)
                if kp > 0:
                    y_kp(ps_y0, 0, kp - 1)
            y_kp(ps_y0, 0, KP - 1)

            def y_mtile(m, pool):
                ps_y = pool.tile([P, HW], F32, tag="mm", name=f"ps_y{s}_{m}")
                for kp in range(KP):
                    y_kp(ps_y, m, kp)
                return ps_y

            ps_y1 = y_mtile(1, psum_rot)

            # Z after y1: its last k-pair needs the final exp, which ACT
            # delivers about now
            ps_z = psum_rot.tile([P, HW], F32, tag="mm", name=f"ps_z{s}")
            for kp in range(KP):
                z_kp(ps_z, kp)

            rzb_sb = rz_pool.tile([P, HW], F32, tag="rz")
            nc.vector.reciprocal_approx_fast(out=rzb_sb[:, 0:512], in_=ps_z[:, 0:512])
            nc.vector.reciprocal_approx_fast(out=rzb_sb[:, 512:HW], in_=ps_z[:, 512:HW])

            def combine(m, ps_y):
                # mul on DVE (psum src); add in bf16 (2x DVE mode); emitted
                # so it pipelines behind the next y-mtile's matmuls
                t1 = fin_pool.tile([P, HW], BF16, tag="fin", name=f"t1_{s}_{m}")
                o_sb = out_pool.tile([P, HW], BF16, tag="o", name=f"o_{s}_{m}")
                nc.vector.tensor_mul(t1[:], ps_y[:], rzb_sb[:])
                nc.vector.tensor_add(o_sb[:], t1[:], xres_sb[:, m, :])
                nc.sync.dma_start(
                    out_d[s].rearrange("(t p) f -> t p f", p=P)[m],
                    o_sb[:],
                )

            combine(0, ps_y0)
            ps_y2 = y_mtile(2, psum_rot)
            combine(1, ps_y1)
            ps_y3 = y_mtile(3, psum_y)
            combine(2, ps_y2)
            combine(3, ps_y3)


_CACHE = {}


def _build():
    if "nc" not in _CACHE:
        nc = bacc.Bacc("TRN2", target_bir_lowering=False, debug=False)
        with tile.TileContext(nc) as tc:
            _emit(tc)
        nc.compile()
        _CACHE["nc"] = nc
    return _CACHE["nc"]


def _prep_in_maps(x, W_theta, b_theta, W_phi, b_phi, W_fuse, b_fuse):
    bf = ml_dtypes.bfloat16
    f8 = ml_dtypes.float8_e4m3
    xf = np.ascontiguousarray(x.reshape(N, C, HW).astype(np.float32))
    x_f8 = xf.astype(f8)
    x_res = (xf + b_fuse.astype(np.float32)[None, :, None]).astype(bf)
    wcat_t = np.ascontiguousarray(
        np.concatenate([W_theta.astype(np.float32) * TP_BOOST,
                        W_phi.astype(np.float32) * TP_BOOST], axis=0).T
    ).astype(f8)
    b_cat = np.concatenate([b_theta.astype(np.float32) * TP_BOOST,
                            b_phi.astype(np.float32) * TP_BOOST]).reshape(2 * D, 1)
    wfu_t = np.ascontiguousarray(
        W_fuse.astype(np.float32).T * FU_BOOST
    ).astype(f8)

    in_maps = []
    for c in range(NCORES):
        sl = slice(c * NS, (c + 1) * NS)
        in_maps.append({
            "x_f8": np.ascontiguousarray(x_f8[sl]),
            "x_res": np.ascontiguousarray(x_res[sl]),
            "wcat_t": wcat_t,
            "b_cat": b_cat.astype(np.float32),
            "wfu_t": wfu_t,
        })
    return in_maps


def _run(inputs, trace=False, **kw):
    nc = _build()
    in_maps = _prep_in_maps(**inputs)
    res = bass_utils.run_bass_kernel_spmd(
        nc, in_maps, core_ids=list(range(NCORES)), trace=trace, **kw
    )
    out = np.concatenate(
        [res.results[c]["out"].astype(np.float32) for c in range(NCORES)], axis=0
    )
    return out.reshape(N, C, H, W), res


def kernel(**inputs):
    inputs = {k: np.asarray(v) for k, v in inputs.items()}
    out, _ = _run(inputs, trace=False)
    return out


# revision 19
# speedup vs baseline: 1.5640x; 1.0372x over previous
"""Non-local (spatial self-attention) denoising block on 8 Trainium2 cores.

Reference math (per sample n, with x:[C,HW], D=C/2):
    t = (W_theta @ x + b_theta) / sqrt(D)      [D, HW]
    p = W_phi   @ x + b_phi                    [D, HW]
    S[q,k] = t[:,q] . p[:,k]
    f = softmax_k(S)
    attn = x @ f.T
    out = x + W_fuse @ attn + b_fuse

Device formulation -- all matmul phases run fp8e4 (e4m3) DoubleRow, which
packs two 128-row k-tiles per instruction (~1.9x bf16 PE throughput):
    Sᵀ[k,q] = p.T @ t    -- keys on partitions so no transposes are needed
    e = exp(Sᵀ/1024 - 3) -- shift is softmax-invariant; keeps e in e4m3 range
    G'ᵀ = xᵀ @ (16·W_fuseᵀ)  -- output conv fused into the values: [HW, C]
    Zb = (16·ones)ᵀ @ e  -- Z on all 128 partitions via accumulating DR
                            matmuls over the 8 e tiles (replaces a 7-op DVE
                            f32 add chain that was the v1 bottleneck)
    y = G'ᵀᵀ @ e          -- [C, HW] unnormalized
    out = y * (1/Zb) + (x + b_fuse)

fp8 scale management (e4m3: max 240, min normal 2^-7): W_theta/W_phi are
boosted x8 host-side (raw 0.05-scale weights would sit in subnormals),
W_fuse x16; the 1/sqrt(D) softmax scale and both theta/phi boosts fold into
the exp activation's scale (1/1024), the W_fuse boost into the Z ones.

Schedule (per sample, steady state): conv -> kp-loop{S(2kp), S(2kp+1),
gt-pair(kp), y0(kp), y1(kp), Z(kp)} -> y2, y3 -> combine. y0/y1 accumulate
in a RESIDENT psum pool across the kp loop so the y matmuls consume e tiles
as ACT produces them -- no exp-lag stall before the y phase. PSUM: resident
pool (y0,y1,y3) 4 banks + rotating pool (conv/S/gt/Z/y2) 4 banks.
Engine balance per sample: PE ~21us, ACT ~11.5us (exps + half the conv
copies), DVE ~16us (other copies, gt copies, recip, final mul/add).

Sharding: data-parallel over batch N=32 -> 4 samples per core on 8 cores.
A ~3.8us burst of junk matmuls pre-warms the PE clock (HAM) while the first
DMAs land; a dummy exp pre-loads the ACT spline table.
"""

import numpy as np
import ml_dtypes

import concourse.bass as bass
import concourse.tile as tile
from concourse import bacc, mybir
from concourse import bass_utils

F32 = mybir.dt.float32
BF16 = mybir.dt.bfloat16
F8 = mybir.dt.float8e4
AF = mybir.ActivationFunctionType
DR = mybir.MatmulPerfMode.DoubleRow

N, C, H, W = 32, 512, 32, 32
D = C // 2
HW = H * W
NCORES = 8
NS = N // NCORES  # samples per core
P = 128
CT = C // P   # 4 c-tiles
KT = HW // P  # 8 hw-tiles
MT_D = (2 * D) // P  # 4 m-tiles of combined theta/phi conv
NQ = HW // 512  # 2 free-dim halves
KP = KT // 2  # 4 k-pairs for DoubleRow over HW

TP_BOOST = 8.0     # on W_theta and W_phi (and their biases)
FU_BOOST = 16.0    # on W_fuse
EXP_SCALE = 1.0 / (TP_BOOST * TP_BOOST * np.sqrt(np.float32(D)))
EXP_BIAS = -3.0    # softmax-shift: keeps exp() within e4m3 range


def _emit(tc):
    nc = tc.nc

    x_f8 = nc.dram_tensor("x_f8", [NS, C, HW], F8, kind="ExternalInput").ap()
    x_res = nc.dram_tensor("x_res", [NS, C, HW], BF16, kind="ExternalInput").ap()
    wcat_t = nc.dram_tensor("wcat_t", [C, 2 * D], F8, kind="ExternalInput").ap()
    b_cat = nc.dram_tensor("b_cat", [2 * D, 1], F32, kind="ExternalInput").ap()
    wfu_t = nc.dram_tensor("wfu_t", [C, C], F8, kind="ExternalInput").ap()
    out_d = nc.dram_tensor("out", [NS, C, HW], BF16, kind="ExternalOutput").ap()

    import contextlib
    ctx = contextlib.ExitStack()
    with ctx:
        # ---- constant pools ----
        wpool = ctx.enter_context(tc.tile_pool(name="wpool", bufs=1))
        wcat_sb = wpool.tile([P, CT, 2 * D], F8)

        # ---- working pools ----
        xf8_pool = ctx.enter_context(tc.tile_pool(name="xf8", bufs=3))
        xres_pool = ctx.enter_context(tc.tile_pool(name="xres", bufs=2))
        tp_pool = ctx.enter_context(tc.tile_pool(name="tp", bufs=2))
        gt_pool = ctx.enter_context(tc.tile_pool(name="gt", bufs=2))
        e_pool = ctx.enter_context(tc.tile_pool(name="e", bufs=2))
        rz_pool = ctx.enter_context(tc.tile_pool(name="rz", bufs=2))
        fin_pool = ctx.enter_context(tc.tile_pool(name="fin", bufs=3))
        out_pool = ctx.enter_context(tc.tile_pool(name="outp", bufs=3))

        # PSUM: 1-buf resident pool for the kp-loop-accumulating y0 (+y3),
        # 3-buf rotating pool for everything else. 2 + 3x2 banks = all 8.
        psum_y = ctx.enter_context(tc.tile_pool(name="psy", bufs=1, space="PSUM"))
        psum_rot = ctx.enter_context(tc.tile_pool(name="psr", bufs=3, space="PSUM"))

        # HAM pre-warm + ACT exp-table pre-load during the initial DMA wait
        ones_f8 = wpool.tile([P, 2, P], F8)
        nc.vector.memset(ones_f8[:], FU_BOOST)  # folds the wfu boost out of Z
        warm_rhs = wpool.tile([P, 512], BF16)
        nc.vector.memset(warm_rhs[:], 0.0)
        ebias_sb = wpool.tile([P, 1], F32)
        nc.vector.memset(ebias_sb[:], EXP_BIAS)
        warm_e = wpool.tile([P, 16], F8)
        nc.scalar.activation(warm_e[:], warm_rhs[:, 0:16], AF.Exp,
                             bias=ebias_sb[:], scale=float(EXP_SCALE))
        ps_warm = psum_rot.tile([P, 512], F32, tag="mm", name="ps_warm")
        warm_lhs = wpool.tile([P, P], BF16)
        nc.vector.memset(warm_lhs[:], 0.0)
        for w in range(9):
            # accumulate: with start=True each, walrus dead-stores 8 of them
            nc.tensor.matmul(ps_warm[:], warm_lhs[:], warm_rhs[:],
                             start=(w == 0), stop=(w == 8))
        # read ps_warm so walrus can't dead-code-eliminate the HAM warmup
        warm_sink = wpool.tile([P, 8], F32)
        nc.vector.tensor_copy(warm_sink[:], ps_warm[:, 0:8])

        xf8_tiles = {}
        xf8_tiles[0] = xf8_pool.tile([P, CT, HW], F8, tag="xf8", name="xf80")
        for k in range(CT):
            nc.sync.dma_start(
                wcat_sb[:, k, :],
                wcat_t.rearrange("(t p) d -> t p d", p=P)[k],
            )
            nc.gpsimd.dma_start(
                xf8_tiles[0][:, k, :],
                x_f8[0].rearrange("(t p) f -> t p f", p=P)[k],
            )

        # bcat (tiny, needed by the first conv copies) first, then wfu on the
        # sync ring right behind wcat so sample 0's gt matmuls don't wait
        bcat_sb = wpool.tile([P, MT_D], F32)
        nc.sync.dma_start(
            bcat_sb.rearrange("p (t o) -> p t o", o=1),
            b_cat.rearrange("(t p) o -> p t o", p=P),
        )
        wfu_sb = wpool.tile([P, CT, C], F8)
        nc.sync.dma_start(
            wfu_sb[:],
            wfu_t.rearrange("(t p) d -> p t d", p=P),
        )
        for s in range(NS):
            # ---- load x (fp8 for matmuls, bf16 residual w/ b_fuse folded) ----
            if s not in xf8_tiles:
                xf8_tiles[s] = xf8_pool.tile(
                    [P, CT, HW], F8, tag="xf8", name=f"xf8{s}"
                )
                nc.sync.dma_start(
                    xf8_tiles[s][:],
                    x_f8[s].rearrange("(t p) f -> p t f", p=P),
                )
            xf8_sb = xf8_tiles[s]
            xres_sb = xres_pool.tile([P, CT, HW], BF16, tag="xres")
            nc.sync.dma_start(
                xres_sb[:],
                x_res[s].rearrange("(t p) f -> p t f", p=P),
            )

            # ---- combined theta/phi 1x1 conv: tp = wcat.T @ x + b ----
            tp_sb = tp_pool.tile([P, MT_D, HW], F8, tag="tp")

            def conv_mm(ps, m, kp):
                for nq in range(NQ):
                    nc.tensor.matmul(
                        ps[:, nq * 512:(nq + 1) * 512],
                        wcat_sb[:, 2 * kp:2 * kp + 2, m * P:(m + 1) * P],
                        xf8_sb[:, 2 * kp:2 * kp + 2, nq * 512:nq * 512 + 512],
                        start=(kp == 0),
                        stop=(kp == CT // 2 - 1),
                        perf_mode=DR,
                    )

            def conv_copy(ps, m):
                # bias-add + fp8 convert on ACT (Identity w/ per-partition
                # bias): ACT is idle at conv time, while DVE is still
                # draining the previous sample's combine ops
                nc.scalar.add(tp_sb[:, m, :], ps[:], bcat_sb[:, m:m + 1])

            if s == 0:
                # k-outer so each arriving x k-pair feeds all m immediately;
                # 4 live psums span both pools
                ps_cvs = [
                    (psum_y if m < 1 else psum_rot).tile(
                        [P, HW], F32, tag="mm", name=f"ps_cv0_{m}")
                    for m in range(MT_D)
                ]
                for kp in range(CT // 2):
                    for m in range(MT_D):
                        conv_mm(ps_cvs[m], m, kp)
                for m in range(MT_D):
                    conv_copy(ps_cvs[m], m)
            else:
                for m in range(MT_D):
                    ps_cv = psum_rot.tile(
                        [P, HW], F32, tag="mm", name=f"ps_cv{s}_{m}"
                    )
                    for kp in range(CT // 2):
                        conv_mm(ps_cv, m, kp)
                    conv_copy(ps_cv, m)

            gt_sb = gt_pool.tile([P, KT, C], F8, tag="gt")
            e_sb = e_pool.tile([P, KT, HW], F8, tag="e")

            def s_mtile(m):
                ps_s = psum_rot.tile([P, HW], F32, tag="mm", name=f"ps_s{s}_{m}")
                for nq in range(NQ):
                    nc.tensor.matmul(
                        ps_s[:, nq * 512:(nq + 1) * 512],
                        tp_sb[:, 2:4, m * P:(m + 1) * P],
                        tp_sb[:, 0:2, nq * 512:nq * 512 + 512],
                        start=True,
                        stop=True,
                        perf_mode=DR,
                    )
                nc.scalar.activation(
                    e_sb[:, m, :], ps_s[:], AF.Exp,
                    bias=ebias_sb[:], scale=float(EXP_SCALE),
                )

            def gt_pair(j):
                # two G'T m-tiles share one psum; single DVE copy (DVE is
                # otherwise idle during the kp loop)
                ps_g = psum_rot.tile([P, HW], F32, tag="mm", name=f"ps_g{s}_{j}")
                for mi in range(2):
                    m = 2 * j + mi
                    for kp in range(CT // 2):
                        nc.tensor.matmul(
                            ps_g[:, mi * C:(mi + 1) * C],
                            xf8_sb[:, 2 * kp:2 * kp + 2, m * P:(m + 1) * P],
                            wfu_sb[:, 2 * kp:2 * kp + 2, :],
                            start=(kp == 0),
                            stop=(kp == CT // 2 - 1),
                            perf_mode=DR,
                        )
                nc.vector.tensor_copy(gt_sb[:, 2 * j:2 * j + 2, :], ps_g[:])

            def y_kp(ps_y, m, kp):
                for nq in range(NQ):
                    nc.tensor.matmul(
                        ps_y[:, nq * 512:(nq + 1) * 512],
                        gt_sb[:, 2 * kp:2 * kp + 2, m * P:(m + 1) * P],
                        e_sb[:, 2 * kp:2 * kp + 2, nq * 512:nq * 512 + 512],
                        start=(kp == 0),
                        stop=(kp == KP - 1),
                        perf_mode=DR,
                        skip_group_check=True,
                    )

            def z_kp(ps_z, kp):
                for nq in range(NQ):
                    nc.tensor.matmul(
                        ps_z[:, nq * 512:(nq + 1) * 512],
                        ones_f8[:],
                        e_sb[:, 2 * kp:2 * kp + 2, nq * 512:nq * 512 + 512],
                        start=(kp == 0),
                        stop=(kp == KP - 1),
                        perf_mode=DR,
                        skip_group_check=True,
                    )

            # kp-loop: gt FIRST in each slot (its psum-slot WAR wait and its
            # DVE CAST then get a full kp of slack before y0 consumes them);
            # y0 consumes the PREVIOUS kp's e/gt tiles (one-step software
            # pipeline) so it never waits on this kp's exp or gt copy
            ps_y0 = psum_y.tile([P, HW], F32, tag="mm", name=f"ps_y{s}_0")
            for kp in range(KP):
                gt_pair(kp)
                s_mtile(2 * kp)
                s_mtile(2 * kp + 1)
                if kp > 0:
                    y_kp(ps_y0, 0, kp - 1)
            y_kp(ps_y0, 0, KP - 1)

            def y_mtile(m, pool):
                ps_y = pool.tile([P, HW], F32, tag="mm", name=f"ps_y{s}_{m}")
                for kp in range(KP):
                    y_kp(ps_y, m, kp)
                return ps_y

            # Z immediately: its last k-pair needs only the final exp, which
            # ACT delivers ~1.3us after the loop -- reached at just that time.
            # Early Z un-gates recip -> mul(y0) -> y3's psum slot.
            ps_z = psum_rot.tile([P, HW], F32, tag="mm", name=f"ps_z{s}")
            for kp in range(KP):
                z_kp(ps_z, kp)

            ps_y1 = y_mtile(1, psum_rot)

            rzb_sb = rz_pool.tile([P, HW], F32, tag="rz")
            nc.vector.reciprocal_approx_fast(out=rzb_sb[:], in_=ps_z[:])

            def combine(m, ps_y):
                # mul on DVE (psum src); add in bf16 (2x DVE mode); emitted
                # so it pipelines behind the next y-mtile's matmuls
                t1 = fin_pool.tile([P, HW], BF16, tag="fin", name=f"t1_{s}_{m}")
                o_sb = out_pool.tile([P, HW], BF16, tag="o", name=f"o_{s}_{m}")
                nc.vector.tensor_mul(t1[:], ps_y[:], rzb_sb[:])
                nc.vector.tensor_add(o_sb[:], t1[:], xres_sb[:, m, :])
                nc.sync.dma_start(
                    out_d[s].rearrange("(t p) f -> t p f", p=P)[m],
                    o_sb[:],
                )

            combine(0, ps_y0)
            ps_y2 = y_mtile(2, psum_rot)
            combine(1, ps_y1)
            ps_y3 = y_mtile(3, psum_y)
            combine(2, ps_y2)
            combine(3, ps_y3)


_CACHE = {}


def _build():
    if "nc" not in _CACHE:
        nc = bacc.Bacc("TRN2", target_bir_lowering=False, debug=False)
        with tile.TileContext(nc) as tc:
            _emit(tc)
        nc.compile()
        _CACHE["nc"] = nc
    return _CACHE["nc"]


def _prep_in_maps(x, W_theta, b_theta, W_phi, b_phi, W_fuse, b_fuse):
    bf = ml_dtypes.bfloat16
    f8 = ml_dtypes.float8_e4m3
    xf = np.ascontiguousarray(x.reshape(N, C, HW).astype(np.float32))
    x_f8 = xf.astype(f8)
    x_res = (xf + b_fuse.astype(np.float32)[None, :, None]).astype(bf)
    wcat_t = np.ascontiguousarray(
        np.concatenate([W_theta.astype(np.float32) * TP_BOOST,
                        W_phi.astype(np.float32) * TP_BOOST], axis=0).T
    ).astype(f8)
    b_cat = np.concatenate([b_theta.astype(np.float32) * TP_BOOST,
                            b_phi.astype(np.float32) * TP_BOOST]).reshape(2 * D, 1)
    wfu_t = np.ascontiguousarray(
        W_fuse.astype(np.float32).T * FU_BOOST
    ).astype(f8)

    in_maps = []
    for c in range(NCORES):
        sl = slice(c * NS, (c + 1) * NS)
        in_maps.append({
            "x_f8": np.ascontiguousarray(x_f8[sl]),
            "x_res": np.ascontiguousarray(x_res[sl]),
            "wcat_t": wcat_t,
            "b_cat": b_cat.astype(np.float32),
            "wfu_t": wfu_t,
        })
    return in_maps


def _run(inputs, trace=False, **kw):
    nc = _build()
    in_maps = _prep_in_maps(**inputs)
    res = bass_utils.run_bass_kernel_spmd(
        nc, in_maps, core_ids=list(range(NCORES)), trace=trace, **kw
    )
    out = np.concatenate(
        [res.results[c]["out"].astype(np.float32) for c in range(NCORES)], axis=0
    )
    return out.reshape(N, C, H, W), res


def kernel(**inputs):
    inputs = {k: np.asarray(v) for k, v in inputs.items()}
    out, _ = _run(inputs, trace=False)
    return out
